# revision 3
# baseline (speedup 1.0000x reference)
"""VQ EuclideanCodebook forward + EMA update on 8 Trainium2 NeuronCores.

Strategy (data-parallel over the 32768 flattened tokens, 4096/core):
  Phase A (per 128-token tile): dist = x@e.T - 0.5||e||^2 via fp32r matmuls
    (PE full rate), PSUM chunks copied to a bf16 [128,8192] row buffer (ACT),
    argmax via DVE Max/MaxIndex (top-8 candidates kept), quantize rows
    gathered from DRAM embed via indirect DMA.
  Phase B: embed_sum = onehot.T @ x as fp32r matmuls, 8 PSUM banks accumulate
    over all 32 token-tiles per 1024-code pass; onehot built by DVE
    iota==idx compare + ACT cast.
  Phase C: ReduceScatter(embed_sum) across the 8 cores, then EMA update of
    this core's 1024-row slice of embed_avg.
  Host: exact fp64 rescore of the top-8 candidates fixes any argmax flips
    from fp32r rounding (the dataset's min top1-top2 gap is ~1e-4; fp32r
    noise ~2e-2), patches the few affected rows, computes the cheap
    8192-element tail math (counts/cluster_size/normalize).
"""

import numpy as np

CB, D = 8192, 512
N_CORES = 8
NT = 32            # 128-token tiles per core
TOK_PER_CORE = NT * 128
DECAY = 0.8
EPS = 1e-05
CB_SLICE = CB // N_CORES   # codes owned per core after reduce-scatter

_NC_CACHE = {}


def _build_nc():
    import concourse.bacc as bacc
    import concourse.mybir as mybir
    import concourse.tile as tile
    from concourse.bass import IndirectOffsetOnAxis

    f32 = mybir.dt.float32
    f32r = mybir.dt.float32r
    bf16 = mybir.dt.bfloat16
    u16 = mybir.dt.uint16
    u32 = mybir.dt.uint32
    f16 = mybir.dt.float16
    Copy = mybir.ActivationFunctionType.Copy

    nc = bacc.Bacc(None, target_bir_lowering=False, debug=False)

    xT_d = nc.dram_tensor("xT", [D, TOK_PER_CORE], f32r, kind="ExternalInput")
    x_d = nc.dram_tensor("x", [TOK_PER_CORE, D], f32r, kind="ExternalInput")
    et_d = nc.dram_tensor("et", [D, CB], f32r, kind="ExternalInput")
    enb_d = nc.dram_tensor("enb", [1, CB], f16, kind="ExternalInput")
    ones_d = nc.dram_tensor("ones", [1, 128], f16, kind="ExternalInput")
    embed_d = nc.dram_tensor("embed", [CB, D], f32, kind="ExternalInput")
    eavg_d = nc.dram_tensor("eavg", [CB_SLICE, D], f32, kind="ExternalInput")

    top8_d = nc.dram_tensor("top8", [TOK_PER_CORE, 8], u16, kind="ExternalOutput")
    quant_d = nc.dram_tensor("quant", [TOK_PER_CORE, D], f32, kind="ExternalOutput")
    eavgn_d = nc.dram_tensor("eavgn", [CB_SLICE, D], f32, kind="ExternalOutput")

    esum_local = nc.dram_tensor("esum_local", [CB, D], f32)
    esum_red = nc.dram_tensor("esum_red", [CB_SLICE, D], f32)

    et_c = et_d.rearrange("(c p) n -> c p n", p=128)          # 4 k-chunks
    xT_c = xT_d.rearrange("(c p) (i m) -> c p i m", p=128, m=128)
    x_t = x_d.rearrange("(i p) d -> p i d", p=128)

    with tile.TileContext(nc) as tc:
        with tc.tile_pool(name="keep", bufs=1) as keep:
            ones_t = keep.tile([1, 128], f16)
            enb_t = keep.tile([1, CB], f16)
            idxf_all = keep.tile([128, NT], f32)
            nc.sync.dma_start(ones_t[:], ones_d[:])
            nc.sync.dma_start(enb_t[:], enb_d[:])

            # ---------------- Phase A ----------------
            with (
                tc.tile_pool(name="etp", bufs=1) as etp,
                tc.tile_pool(name="pa", bufs=2) as pa,
                tc.tile_pool(name="psa", bufs=8, space="PSUM") as psa,
            ):
                et_t = etp.tile([128, 4, CB], f32r)
                for c in range(4):
                    nc.sync.dma_start(et_t[:, c, :], et_c[c])

                for i in range(NT):
                    xt = pa.tile([128, 4, 128], f32r, name="xt")
                    for c in range(4):
                        nc.sync.dma_start(xt[:, c, :], xT_c[c, :, i, :])
                    dist = pa.tile([128, CB], bf16, name="dist")
                    for n in range(16):
                        ps = psa.tile([128, 512], f32, name="ps")
                        nc.tensor.matmul(ps[:], ones_t[:],
                                         enb_t[:, n * 512:(n + 1) * 512],
                                         start=True, stop=False)
                        for c in range(4):
                            nc.tensor.matmul(
                                ps[:], xt[:, c, :],
                                et_t[:, c, n * 512:(n + 1) * 512],
                                start=False, stop=(c == 3))
                        nc.scalar.activation(
                            dist[:, n * 512:(n + 1) * 512], ps[:], Copy)
                    mx8 = pa.tile([128, 8], bf16, name="mx8")
                    ix8 = pa.tile([128, 8], u16, name="ix8")
                    nc.vector.max(mx8[:], dist[:])
                    nc.vector.max_index(ix8[:], mx8[:], dist[:])
                    nc.sync.dma_start(top8_d[i * 128:(i + 1) * 128, :], ix8[:])
                    nc.vector.tensor_copy(idxf_all[:, i:i + 1], ix8[:, 0:1])
                    gidx = pa.tile([128, 1], u32, name="gidx")
                    nc.vector.tensor_copy(gidx[:], ix8[:, 0:1])
                    gth = pa.tile([128, D], f32, name="gth")
                    nc.gpsimd.indirect_dma_start(
                        gth[:], None, embed_d[:],
                        IndirectOffsetOnAxis(ap=gidx[:], axis=0))
                    nc.sync.dma_start(quant_d[i * 128:(i + 1) * 128, :], gth[:])

            # ---------------- Phase B ----------------
            with (
                tc.tile_pool(name="pbx", bufs=1) as pbx,
                tc.tile_pool(name="pb", bufs=3) as pb,
                tc.tile_pool(name="psb", bufs=1, space="PSUM") as psb,
            ):
                xres = pbx.tile([128, NT, D], f32r)
                nc.sync.dma_start(xres[:], x_t)
                iota_t = pbx.tile([128, CB], u16)
                nc.gpsimd.iota(iota_t[:], pattern=[[1, CB]], base=0,
                               channel_multiplier=0)
                for p in range(8):
                    pss = [psb.tile([128, D], f32, name=f"pse{j}") for j in range(8)]
                    for t in range(NT):
                        ohb = pb.tile([128, 1024], bf16, name="ohb")
                        nc.vector.tensor_scalar(
                            ohb[:], iota_t[:, p * 1024:(p + 1) * 1024],
                            idxf_all[:, t:t + 1], None,
                            mybir.AluOpType.is_equal)
                        ohf = pb.tile([128, 1024], f32r, name="ohf")
                        nc.scalar.activation(ohf[:], ohb[:], Copy)
                        for j in range(8):
                            nc.tensor.matmul(
                                pss[j][:], ohf[:, j * 128:(j + 1) * 128],
                                xres[:, t, :],
                                start=(t == 0), stop=(t == NT - 1))
                    for j in range(8):
                        stg = pb.tile([128, D], f32, name="stg")
                        nc.scalar.activation(stg[:], pss[j][:], Copy)
                        r0 = (p * 8 + j) * 128
                        nc.sync.dma_start(esum_local[r0:r0 + 128, :], stg[:])

            # ---------------- Phase C ----------------
            import concourse.mybir as mybir2
            nc.gpsimd.collective_compute(
                "ReduceScatter", mybir2.AluOpType.add,
                replica_groups=[list(range(N_CORES))],
                ins=[esum_local[:]], outs=[esum_red[:]])
            with tc.tile_pool(name="pc", bufs=2) as pc:
                for j in range(CB_SLICE // 128):
                    ea = pc.tile([128, D], f32, name="ea")
                    es = pc.tile([128, D], f32, name="es")
                    nc.sync.dma_start(ea[:], eavg_d[j * 128:(j + 1) * 128, :])
                    nc.sync.dma_start(es[:], esum_red[j * 128:(j + 1) * 128, :])
                    t1 = pc.tile([128, D], f32, name="t1")
                    nc.scalar.activation(t1[:], ea[:], Copy, scale=DECAY)
                    t2 = pc.tile([128, D], f32, name="t2")
                    nc.scalar.activation(t2[:], es[:], Copy, scale=1.0 - DECAY)
                    eo = pc.tile([128, D], f32, name="eo")
                    nc.vector.tensor_add(eo[:], t1[:], t2[:])
                    nc.sync.dma_start(eavgn_d[j * 128:(j + 1) * 128, :], eo[:])
    nc.compile()
    return nc


def _get_nc():
    if "nc" not in _NC_CACHE:
        _NC_CACHE["nc"] = _build_nc()
    return _NC_CACHE["nc"]


def _run_device(in_maps):
    from concourse.bass_utils import run_bass_kernel_spmd
    nc = _get_nc()
    return run_bass_kernel_spmd(nc, in_maps, core_ids=list(range(N_CORES))).results


def kernel(x, embed, cluster_size, embed_avg):
    x = np.asarray(x, dtype=np.float32)
    embed = np.asarray(embed, dtype=np.float32)
    cluster_size = np.asarray(cluster_size, dtype=np.float32)
    embed_avg = np.asarray(embed_avg, dtype=np.float32)

    shape = x.shape
    xf = np.ascontiguousarray(x.reshape(-1, D))          # [32768, 512]
    n_tok = xf.shape[0]
    assert n_tok == N_CORES * TOK_PER_CORE

    et = np.ascontiguousarray(embed.T)                   # [512, 8192]
    enb = (-0.5 * (embed.astype(np.float64) ** 2).sum(1)).astype(np.float16)[None, :]
    ones = np.ones((1, 128), np.float16)

    in_maps = []
    for c in range(N_CORES):
        xs = xf[c * TOK_PER_CORE:(c + 1) * TOK_PER_CORE]
        in_maps.append({
            "xT": np.ascontiguousarray(xs.T),
            "x": np.ascontiguousarray(xs),
            "et": et,
            "enb": enb,
            "ones": ones,
            "embed": embed,
            "eavg": np.ascontiguousarray(
                embed_avg[c * CB_SLICE:(c + 1) * CB_SLICE]),
        })

    results = _run_device(in_maps)

    top8 = np.concatenate([r["top8"] for r in results], axis=0)      # [32768, 8] u16
    quant = np.concatenate([r["quant"] for r in results], axis=0)    # [32768, 512]
    eavg_new = np.concatenate([r["eavgn"] for r in results], axis=0)  # [8192, 512]

    # ---- host: exact rescore of the top-8 candidates (fp64) ----
    cand = top8.astype(np.int64)                                     # [N, 8]
    valid = cand < CB
    cand_safe = np.where(valid, cand, 0)
    e64 = embed.astype(np.float64)
    en64 = 0.5 * (e64 ** 2).sum(1)                                   # [8192]
    x64 = xf.astype(np.float64)
    # s[t, j] = x[t] . e[cand[t,j]] - 0.5||e||^2
    ecand = e64[cand_safe]                                           # [N, 8, 512]
    s = np.einsum("td,tjd->tj", x64, ecand) - en64[cand_safe]
    s[~valid] = -np.inf
    # ties -> lowest code index (mirror jnp.argmax first-max semantics)
    smax = s.max(axis=1, keepdims=True)
    cand_masked = np.where(s == smax, cand_safe, CB)
    win = cand_masked.min(axis=1).astype(np.int64)                   # [N]

    dev_choice = cand_safe[:, 0]
    flipped = np.nonzero(win != dev_choice)[0]
    if flipped.size:
        quant[flipped] = embed[win[flipped]]
        old = dev_choice[flipped]
        new = win[flipped]
        np.add.at(eavg_new, old, (-(1.0 - DECAY) * xf[flipped]))
        np.add.at(eavg_new, new, ((1.0 - DECAY) * xf[flipped]))

    counts = np.bincount(win, minlength=CB).astype(np.float32)
    cluster_size_new = (np.float32(DECAY) * cluster_size
                        + np.float32(1.0 - DECAY) * counts)
    ssum = cluster_size_new.sum()
    smoothed = (cluster_size_new + np.float32(EPS)) / (ssum + CB * np.float32(EPS))
    cs = smoothed * ssum
    embed_normalized = eavg_new / cs[:, None]

    quantize = quant.reshape(shape)
    embed_ind = win.astype(np.int32).reshape(shape[:-1])
    return (quantize, embed_ind, cluster_size_new,
            eavg_new.astype(np.float32), embed_normalized.astype(np.float32))
